# revision 11
# baseline (speedup 1.0000x reference)
"""Distributed column-sum-of-squares loss kernel for TRN2 (8 NeuronCores).

Computes 0.001 * || (D^T @ D) * I - I ||_F for D [262144, 512] f32, i.e.
    loss = 0.001 * sqrt( sum_j (||D[:, j]||^2 - 1)^2 )

Strategy (data parallel over rows, per the sharding hint):
  - Shard D row-wise across the 8 cores (32768 rows each, 64 MiB/core).
  - Per core: stream 2 MiB [128, 8*512] f32 chunks from HBM (alternating
    the two HWDGE rings so both ring's SDMA sets pull concurrently at
    the ~430 GB/s SBUF-fabric ceiling).
  - All input DMAs issue from the SP (sync) sequencer, which does
    nothing else: a dma_start stalled on buffer backpressure must never
    sit in front of compute work in a sequencer FIFO (that coupling
    cost 24 us/run in an earlier revision).
  - Square the 8 row-blocks of each chunk split across two engines so
    neither exceeds the 4.9 us chunk DMA period: ACT squares blocks
    0-3 (fp32 in, bf16 out), DVE squares blocks 4-7 and folds its own
    four into one with two bf16 packed-2x adds (no ACT->DVE dependency,
    so a hiccup on one engine cannot stall the other). Five TensorE
    matmuls per chunk against a ones vector accumulate per-column sums
    into a [1, 512] f32 PSUM bank.
  - Each core emits its partial per-column sum of squares [1, 512]; the
    tiny cross-core reduction + norm epilogue runs on host (the [d]
    vector combine the hint's all-reduce would do on-device).
"""

from contextlib import ExitStack

import numpy as np

import concourse.bass as bass
import concourse.tile as tile
from concourse import bacc, mybir
from concourse.bass_utils import run_bass_kernel_spmd

N_CORES = 8
N_ROWS, N_COLS = 262144, 512
ROWS_PER_CORE = N_ROWS // N_CORES  # 32768
P = 128  # SBUF partitions
T = 8  # row-blocks of 128 per chunk -> free dim T*N_COLS = 4096 (2 MiB f32)
# chunk schedule: steady-state 2 MiB chunks, then progressively smaller
# trailing chunks so the final DMA-land -> matmul chain is short
CHUNK_TS = [8] * 30 + [4, 4, 2, 2, 2, 2]
assert sum(CHUNK_TS) * P == ROWS_PER_CORE

_NC_CACHE = {}


def _build_nc():
    nc = bacc.Bacc(
        "TRN2", target_bir_lowering=False, debug=False, num_devices=N_CORES
    )
    d_in = nc.dram_tensor(
        "d_shard", [ROWS_PER_CORE, N_COLS], mybir.dt.float32, kind="ExternalInput"
    ).ap()
    out = nc.dram_tensor(
        "partial", [1, N_COLS], mybir.dt.float32, kind="ExternalOutput"
    ).ap()

    with tile.TileContext(nc) as tc, ExitStack() as ctx:
        in_pool = ctx.enter_context(tc.tile_pool(name="in", bufs=8))
        sq_pool = ctx.enter_context(tc.tile_pool(name="sq", bufs=6))
        psum_pool = ctx.enter_context(tc.tile_pool(name="psum", bufs=1, space="PSUM"))
        const_pool = ctx.enter_context(tc.tile_pool(name="const", bufs=1))
        res_pool = ctx.enter_context(tc.tile_pool(name="res", bufs=1))

        ones = const_pool.tile([P, 1], mybir.dt.bfloat16)
        nc.vector.memset(ones, 1.0)
        psum = psum_pool.tile([1, N_COLS], mybir.dt.float32)

        with nc.allow_low_precision("bf16 partial-sum fold; error ~2^-9/sqrt(128k)"):
            r0 = 0
            for s, tc_blocks in enumerate(CHUNK_TS):
                rows = P * tc_blocks
                # partition p reads a contiguous tc_blocks*512-elem run
                src = d_in[r0 : r0 + rows, :].rearrange(
                    "(p t) d -> p t d", p=P, t=tc_blocks
                )
                r0 += rows
                t_in = in_pool.tile([P, T, N_COLS], mybir.dt.float32)
                # one ring (qSyncDynamicHW) drives all 16 SDMA engines
                nc.sync.dma_start(out=t_in[:, :tc_blocks, :], in_=src)
                sq = sq_pool.tile([P, T, N_COLS], mybir.dt.bfloat16)
                # square split: ACT does the first half, DVE the second
                a = tc_blocks // 2
                nc.scalar.square(sq[:, :a, :], t_in[:, :a, :])
                nc.vector.tensor_mul(
                    sq[:, a:tc_blocks, :],
                    t_in[:, a:tc_blocks, :],
                    t_in[:, a:tc_blocks, :],
                )
                # fold DVE's blocks [a, tc_blocks) down to block a
                d = tc_blocks - a
                while d > 1:
                    h = d // 2
                    nc.vector.tensor_add(
                        sq[:, a : a + h, :],
                        sq[:, a : a + h, :],
                        sq[:, a + h : a + d, :],
                    )
                    d = h
                for t in range(a + 1):
                    # psum[1, 512] += ones[128, 1].T @ sq[:, t, :]
                    nc.tensor.matmul(
                        psum,
                        lhsT=ones,
                        rhs=sq[:, t, :],
                        start=(s == 0 and t == 0),
                        stop=(s == len(CHUNK_TS) - 1 and t == a),
                    )

        res = res_pool.tile([1, N_COLS], mybir.dt.float32)
        nc.vector.tensor_copy(res, psum)
        nc.sync.dma_start(out=out, in_=res)

    nc.compile()
    return nc


def _run_device(D, **spmd_kwargs):
    """Run the per-core partial reduction; returns (partials [8, 512], results)."""
    if "nc" not in _NC_CACHE:
        _NC_CACHE["nc"] = _build_nc()
    nc = _NC_CACHE["nc"]
    D = np.ascontiguousarray(np.asarray(D, dtype=np.float32))
    shards = np.split(D, N_CORES, axis=0)
    in_maps = [{"d_shard": s} for s in shards]
    res = run_bass_kernel_spmd(nc, in_maps, core_ids=list(range(N_CORES)), **spmd_kwargs)
    partials = np.stack([np.asarray(r["partial"]).reshape(N_COLS) for r in res.results])
    return partials, res


def kernel(D):
    partials, _ = _run_device(D)
    total = partials.sum(axis=0, dtype=np.float64)
    resid = total - 1.0
    loss = 0.001 * np.sqrt(np.sum(resid * resid))
    return np.array(loss, dtype=np.float32)
